# revision 4
# baseline (speedup 1.0000x reference)
"""MLA (multi-head latent attention) Bass kernel for Trainium2, 8 NeuronCores.

Sharding: batch (2) x head-group (4 groups of 4 heads) = 8 cores.
Each core computes, for its batch b and head group g:
  - latents  qlatT = (x_b @ wq_down)^T, kvlatT = (x_b @ wkv_down)^T  (replicated per batch)
  - k_rope   (shared across heads, replicated)
  - up-projections for its 4 heads, causal attention, and a partial
    output  out_partial = ctx_g @ wo[512g:512(g+1), :].
Host sums the 4 partial outputs per batch (the wo row-shard reduction).

All matmuls run in bf16 with fp32 PSUM accumulation. Softmax skips the
row-max subtraction (scores are O(+-10), exp stays in fp32 range).
"""
import math
import sys

sys.path.insert(0, "/opt/trn_rl_repo")

import numpy as np
import ml_dtypes

B, L, H = 2, 2048, 2048
NH, HD, RD = 16, 128, 64
QR, KVR = 768, 512
NHC = 4            # heads per core
N_CORES = 8
SCALE = 1.0 / math.sqrt(HD + RD)
BF = ml_dtypes.bfloat16

_NC_CACHE = {}


def build_nc(l_tokens=L):
    """Build + compile the per-core Bass program (parametrized by sequence
    length for small-scale testing; the real kernel uses l_tokens=L)."""
    import concourse.bass as bass  # noqa: F401
    import concourse.tile as tile
    from concourse import bacc, mybir

    dt = mybir.dt
    Lk = l_tokens
    assert Lk % 512 == 0
    TB = Lk // 512          # 512-token blocks
    KC = Lk // 128          # 128-token chunks
    HB = H // 512           # output column blocks

    nc = bacc.Bacc("TRN2", target_bir_lowering=False, debug=False,
                   num_devices=N_CORES)

    def din(name, shape, d=dt.bfloat16):
        return nc.dram_tensor(name, shape, d, kind="ExternalInput").ap()

    xT = din("xT", [H, Lk])
    wqd = din("wqd", [H, QR])
    wkvd = din("wkvd", [H, KVR])
    wkr = din("wkr", [H, RD])
    wqu = din("wqu", [QR, NHC * HD])
    wqr = din("wqr", [QR, NHC * RD])
    wku = din("wku", [KVR, NHC * HD])
    wvu = din("wvu", [KVR, NHC * HD])
    wo = din("wo", [NHC * HD, H])
    cosT = din("cosT", [Lk, RD], dt.float32)
    ssT = din("ssT", [Lk, RD], dt.float32)     # [-sin | +sin]
    maskm = din("maskm", [4, 128, 512])        # multiplicative causal masks
    ident = din("ident", [128, 128])
    out = nc.dram_tensor("out", [Lk, H], dt.float32, kind="ExternalOutput").ap()

    with tile.TileContext(nc) as tc:
        with (
            tc.tile_pool(name="const", bufs=1) as cpool,
            tc.tile_pool(name="attn", bufs=1) as apool,
            tc.tile_pool(name="stream", bufs=2) as spool,
            tc.tile_pool(name="wcolp", bufs=3) as wpool,
            tc.tile_pool(name="ptp", bufs=2) as ptpool,
            tc.tile_pool(name="obp", bufs=2) as opool,
            tc.tile_pool(name="recp", bufs=1) as rpool,
            tc.tile_pool(name="psA", bufs=2, space="PSUM") as psA,      # [128,512] f32
            tc.tile_pool(name="psSmall", bufs=2, space="PSUM") as psS,  # small f32
            tc.tile_pool(name="psTp", bufs=2, space="PSUM") as psT,     # bf16 transposes
            tc.tile_pool(name="psCtx", bufs=1, space="PSUM") as psC,
            tc.tile_pool(name="psSum", bufs=1, space="PSUM") as psM,
        ):
            # ---- constants ----
            wqu_sb = cpool.tile([128, QR // 128, 512], dt.bfloat16, name="wqu_sb")
            nc.sync.dma_start(wqu_sb, wqu.rearrange("(m p) n -> p m n", p=128))
            wqr_sb = cpool.tile([128, QR // 128, 256], dt.bfloat16, name="wqr_sb")
            nc.sync.dma_start(wqr_sb, wqr.rearrange("(m p) n -> p m n", p=128))
            wku_sb = cpool.tile([128, KVR // 128, 512], dt.bfloat16, name="wku_sb")
            nc.sync.dma_start(wku_sb, wku.rearrange("(m p) n -> p m n", p=128))
            wvu_sb = cpool.tile([128, KVR // 128, 512], dt.bfloat16, name="wvu_sb")
            nc.sync.dma_start(wvu_sb, wvu.rearrange("(m p) n -> p m n", p=128))
            wo_sb = cpool.tile([128, NHC, H], dt.bfloat16, name="wo_sb")
            nc.sync.dma_start(wo_sb, wo.rearrange("(h p) n -> p h n", p=128))
            wkr_sb = cpool.tile([128, H // 128, RD], dt.bfloat16, name="wkr_sb")
            nc.sync.dma_start(wkr_sb, wkr.rearrange("(k p) d -> p k d", p=128))
            cos_sb = cpool.tile([128, KC, RD], dt.float32, name="cos_sb")
            nc.sync.dma_start(cos_sb, cosT.rearrange("(c p) d -> p c d", p=128))
            ss_sb = cpool.tile([128, KC, RD], dt.float32, name="ss_sb")
            nc.sync.dma_start(ss_sb, ssT.rearrange("(c p) d -> p c d", p=128))
            mask_sb = cpool.tile([128, 4, 512], dt.bfloat16, name="mask_sb")
            nc.sync.dma_start(mask_sb, maskm.rearrange("m p j -> p m j"))
            id_sb = cpool.tile([128, 128], dt.bfloat16, name="id_sb")
            nc.sync.dma_start(id_sb, ident)
            ones_sb = cpool.tile([128, 1], dt.bfloat16, name="ones_sb")
            nc.vector.memset(ones_sb, 1.0)

            # ---- persistent attention operands ----
            qcT_sb = apool.tile([128, NHC, Lk], dt.bfloat16, name="qcT_sb")
            kcT_sb = apool.tile([128, NHC, Lk], dt.bfloat16, name="kcT_sb")
            qrT_sb = apool.tile([128, 2, Lk], dt.bfloat16, name="qrT_sb")
            krT_sb = apool.tile([128, Lk], dt.bfloat16, name="krT_sb")  # duplicated rows
            v_sb = apool.tile([128, KC, 512], dt.bfloat16, name="v_sb")
            ctxTn_sb = apool.tile([128, NHC, Lk], dt.bfloat16, name="ctxTn_sb")

            # ================= Phase 1: projections =================
            for tb in range(TB):
                ts0 = tb * 512
                xb = spool.tile([128, H // 128, 512], dt.bfloat16, tag="xb")
                nc.sync.dma_start(
                    xb, xT[:, ts0:ts0 + 512].rearrange("(k p) t -> p k t", p=128))

                # q latent (feature-on-partition), bf16
                qlb = spool.tile([128, QR // 128, 512], dt.bfloat16, tag="qlb")
                for m in range(QR // 128):
                    wc = wpool.tile([128, H // 128, 128], dt.bfloat16, tag="wcol")
                    nc.sync.dma_start(
                        wc, wqd[:, m * 128:(m + 1) * 128]
                        .rearrange("(k p) m -> p k m", p=128))
                    ps = psA.tile([128, 512], dt.float32, tag="mm")
                    for k in range(H // 128):
                        nc.tensor.matmul(ps, wc[:, k, :], xb[:, k, :],
                                         start=(k == 0), stop=(k == H // 128 - 1))
                    nc.scalar.copy(qlb[:, m, :], ps)

                # kv latent
                kvb = spool.tile([128, KVR // 128, 512], dt.bfloat16, tag="kvb")
                for m in range(KVR // 128):
                    wc = wpool.tile([128, H // 128, 128], dt.bfloat16, tag="wcol")
                    nc.sync.dma_start(
                        wc, wkvd[:, m * 128:(m + 1) * 128]
                        .rearrange("(k p) m -> p k m", p=128))
                    ps = psA.tile([128, 512], dt.float32, tag="mm")
                    for k in range(H // 128):
                        nc.tensor.matmul(ps, wc[:, k, :], xb[:, k, :],
                                         start=(k == 0), stop=(k == H // 128 - 1))
                    nc.scalar.copy(kvb[:, m, :], ps)

                # qcT: content query, feature-on-partition
                for hc in range(NHC):
                    ps = psA.tile([128, 512], dt.float32, tag="mm")
                    for m in range(QR // 128):
                        nc.tensor.matmul(ps, wqu_sb[:, m, hc * 128:(hc + 1) * 128],
                                         qlb[:, m, :],
                                         start=(m == 0), stop=(m == QR // 128 - 1))
                    nc.scalar.copy(qcT_sb[:, hc, ts0:ts0 + 512], ps)

                # kcT
                for hc in range(NHC):
                    ps = psA.tile([128, 512], dt.float32, tag="mm")
                    for m in range(KVR // 128):
                        nc.tensor.matmul(ps, wku_sb[:, m, hc * 128:(hc + 1) * 128],
                                         kvb[:, m, :],
                                         start=(m == 0), stop=(m == KVR // 128 - 1))
                    nc.scalar.copy(kcT_sb[:, hc, ts0:ts0 + 512], ps)

                # v (token-on-partition)
                for tc2 in range(4):
                    ps = psA.tile([128, 512], dt.float32, tag="mm")
                    for m in range(KVR // 128):
                        nc.tensor.matmul(ps, kvb[:, m, tc2 * 128:(tc2 + 1) * 128],
                                         wvu_sb[:, m, :],
                                         start=(m == 0), stop=(m == KVR // 128 - 1))
                    nc.scalar.copy(v_sb[:, tb * 4 + tc2, :], ps)

                # k_rope and q_rope per 128-token chunk (token layout -> rope ->
                # PE transpose into feature-on-partition)
                for tc2 in range(4):
                    gc = tb * 4 + tc2
                    tsl = slice(tc2 * 128, (tc2 + 1) * 128)

                    kr_ps = psS.tile([128, RD], dt.float32, tag="sm")
                    for k in range(H // 128):
                        nc.tensor.matmul(kr_ps, xb[:, k, tsl], wkr_sb[:, k, :],
                                         start=(k == 0), stop=(k == H // 128 - 1))
                    t1 = spool.tile([128, RD], dt.float32, tag="t1")
                    nc.vector.tensor_tensor(t1, kr_ps, cos_sb[:, gc, :],
                                            mybir.AluOpType.mult)
                    t2 = spool.tile([128, RD], dt.float32, tag="t2")
                    nc.vector.tensor_tensor(t2[:, 0:32], kr_ps[:, 32:64],
                                            ss_sb[:, gc, 0:32], mybir.AluOpType.mult)
                    nc.vector.tensor_tensor(t2[:, 32:64], kr_ps[:, 0:32],
                                            ss_sb[:, gc, 32:64], mybir.AluOpType.mult)
                    krb = spool.tile([128, 128], dt.bfloat16, tag="krb")
                    nc.vector.tensor_tensor(krb[:, 0:64], t1, t2,
                                            mybir.AluOpType.add)
                    nc.vector.tensor_copy(krb[:, 64:128], krb[:, 0:64])
                    ktp = psT.tile([128, 128], dt.bfloat16, tag="tp")
                    nc.tensor.transpose(ktp, krb, id_sb)
                    nc.vector.tensor_copy(krT_sb[:, gc * 128:(gc + 1) * 128], ktp)

                    qr_ps = psS.tile([128, NHC * RD], dt.float32, tag="sm")
                    for m in range(QR // 128):
                        nc.tensor.matmul(qr_ps, qlb[:, m, tsl], wqr_sb[:, m, :],
                                         start=(m == 0), stop=(m == QR // 128 - 1))
                    qrv = qr_ps.rearrange("p (h d) -> p h d", d=RD)
                    q1 = spool.tile([128, NHC, RD], dt.float32, tag="q1")
                    nc.vector.tensor_tensor(
                        q1, qrv,
                        cos_sb[:, gc, None, :].to_broadcast([128, NHC, RD]),
                        mybir.AluOpType.mult)
                    q2 = spool.tile([128, NHC, RD], dt.float32, tag="q2")
                    nc.vector.tensor_tensor(
                        q2[:, :, 0:32], qrv[:, :, 32:64],
                        ss_sb[:, gc, None, 0:32].to_broadcast([128, NHC, 32]),
                        mybir.AluOpType.mult)
                    nc.vector.tensor_tensor(
                        q2[:, :, 32:64], qrv[:, :, 0:32],
                        ss_sb[:, gc, None, 32:64].to_broadcast([128, NHC, 32]),
                        mybir.AluOpType.mult)
                    qrb = spool.tile([128, NHC * RD], dt.bfloat16, tag="qrb")
                    nc.vector.tensor_tensor(
                        qrb.rearrange("p (h d) -> p h d", d=RD), q1, q2,
                        mybir.AluOpType.add)
                    for hp in range(2):
                        qtp = psT.tile([128, 128], dt.bfloat16, tag="tp")
                        nc.tensor.transpose(qtp, qrb[:, hp * 128:(hp + 1) * 128],
                                            id_sb)
                        nc.vector.tensor_copy(qrT_sb[:, hp, gc * 128:(gc + 1) * 128],
                                              qtp)

            # ================= Phase 2: attention =================
            for s in range(TB):
                qsl = slice(s * 512, (s + 1) * 512)
                for h in range(NHC):
                    hp, half = divmod(h, 2)
                    base = 64 * half
                    nck = 4 * s + 4
                    ctx_ps = psC.tile([128, 512], dt.float32, tag="ctx")
                    sum_ps = psM.tile([1, 512], dt.float32, tag="sum")
                    for c in range(nck):
                        ksl = slice(c * 128, (c + 1) * 128)
                        sc = psA.tile([128, 512], dt.float32, tag="mm")
                        nc.tensor.matmul(sc, kcT_sb[:, h, ksl], qcT_sb[:, h, qsl],
                                         start=True, stop=False)
                        nc.tensor.matmul(
                            sc,
                            krT_sb[base:base + 64, ksl],
                            qrT_sb[base:base + 64, hp, qsl],
                            start=False, stop=True)
                        pt = ptpool.tile([128, 512], dt.bfloat16, tag="pt")
                        nc.scalar.activation(pt, sc,
                                             mybir.ActivationFunctionType.Exp,
                                             scale=SCALE)
                        if c // 4 == s:
                            nc.vector.tensor_tensor(pt, pt, mask_sb[:, c % 4, :],
                                                    mybir.AluOpType.mult)
                        nc.tensor.matmul(sum_ps, ones_sb, pt,
                                         start=(c == 0), stop=(c == nck - 1))
                        nc.tensor.matmul(ctx_ps, v_sb[:, c, h * 128:(h + 1) * 128],
                                         pt, start=(c == 0), stop=(c == nck - 1))
                    rec = rpool.tile([1, 512], dt.float32, tag="rec")
                    nc.vector.reciprocal(rec, sum_ps)
                    rb = rpool.tile([128, 512], dt.float32, tag="rb")
                    nc.gpsimd.partition_broadcast(rb, rec)
                    nc.vector.tensor_tensor(ctxTn_sb[:, h, qsl], ctx_ps, rb,
                                            mybir.AluOpType.mult)

            # ================= Phase 3: output projection =================
            for tc3 in range(KC):
                csl = slice(tc3 * 128, (tc3 + 1) * 128)
                for nb in range(HB):
                    nsl = slice(nb * 512, (nb + 1) * 512)
                    po = psA.tile([128, 512], dt.float32, tag="mm")
                    for h in range(NHC):
                        nc.tensor.matmul(po, ctxTn_sb[:, h, csl], wo_sb[:, h, nsl],
                                         start=(h == 0), stop=(h == NHC - 1))
                    ob = opool.tile([128, 512], dt.float32, tag="ob")
                    nc.vector.tensor_copy(ob, po)
                    nc.sync.dma_start(out[csl, nsl], ob)

    nc.compile()
    return nc


def _host_tables(l_tokens):
    inv_freq = (1.0 / (10000.0 ** (np.arange(0, RD, 2, dtype=np.float32) / RD))
                ).astype(np.float32)
    pos = np.arange(l_tokens, dtype=np.float32)
    freqs = np.outer(pos, inv_freq).astype(np.float32)
    cos_t = np.concatenate([np.cos(freqs), np.cos(freqs)], axis=-1)
    ss_t = np.concatenate([-np.sin(freqs), np.sin(freqs)], axis=-1)
    return cos_t.astype(np.float32), ss_t.astype(np.float32)


def _host_masks():
    r = np.arange(128)[:, None]
    j = np.arange(512)[None, :]
    m = np.stack([(j >= 128 * mm + r) for mm in range(4)]).astype(np.float32)
    return m.astype(BF)


def make_in_maps(inputs, l_tokens=L):
    """Build the 8 per-core input maps from the full (unsharded) inputs."""
    x = np.asarray(inputs["x"], np.float32)
    cos_t, ss_t = _host_tables(l_tokens)
    maskm = _host_masks()
    ident = np.eye(128, dtype=np.float32).astype(BF)

    xTs = [np.ascontiguousarray(x[b, :l_tokens].T).astype(BF) for b in range(x.shape[0])]
    wqd = np.asarray(inputs["wq_down"], np.float32).astype(BF)
    wkvd = np.asarray(inputs["wkv_down"], np.float32).astype(BF)
    wkr = np.asarray(inputs["wk_rope"], np.float32).astype(BF)
    wqu = np.asarray(inputs["wq_up"], np.float32).astype(BF)
    wqr = np.asarray(inputs["wq_rope"], np.float32).astype(BF)
    wku = np.asarray(inputs["wk_up"], np.float32).astype(BF)
    wvu = np.asarray(inputs["wv_up"], np.float32).astype(BF)
    wo = np.asarray(inputs["wo"], np.float32).astype(BF)

    in_maps = []
    for core in range(N_CORES):
        b, g = divmod(core, 4)
        in_maps.append({
            "xT": xTs[b],
            "wqd": wqd,
            "wkvd": wkvd,
            "wkr": wkr,
            "wqu": np.ascontiguousarray(wqu[:, g * 512:(g + 1) * 512]),
            "wqr": np.ascontiguousarray(wqr[:, g * 256:(g + 1) * 256]),
            "wku": np.ascontiguousarray(wku[:, g * 512:(g + 1) * 512]),
            "wvu": np.ascontiguousarray(wvu[:, g * 512:(g + 1) * 512]),
            "wo": np.ascontiguousarray(wo[g * 512:(g + 1) * 512, :]),
            "cosT": cos_t,
            "ssT": ss_t,
            "maskm": maskm,
            "ident": ident,
        })
    return in_maps


def kernel(**inputs):
    from concourse.bass_utils import run_bass_kernel_spmd

    if L not in _NC_CACHE:
        _NC_CACHE[L] = build_nc(L)
    nc = _NC_CACHE[L]
    in_maps = make_in_maps(inputs, L)
    res = run_bass_kernel_spmd(nc, in_maps, list(range(N_CORES)))
    out = np.zeros((B, L, H), np.float32)
    for core in range(N_CORES):
        b, _g = divmod(core, 4)
        out[b] += res.results[core]["out"]
    return out
